# revision 30
# baseline (speedup 1.0000x reference)
"""Trainium2 Bass kernel for causal multi-head attention with RoPE.

Full-input contract: kernel(**inputs) takes the unsharded tensors and
returns the full [B, S, D] output. Internally the work is sharded over
8 NeuronCores: cores 0-3 compute batch 0, cores 4-7 batch 1; within a
batch group each core owns 4 of the 16 heads (tensor-parallel over
heads). Each core computes its partial output-projection contribution
[S, D]; the host sums the 4 partials per batch and adds the biases
that commute with attention (wo_b, and wv_b which passes through the
softmax untouched because attention weights sum to 1).

The on-device data path is bf16 (fp32 PSUM accumulation): same PE
rate as fp32r (1 row/cycle) but half the DMA/SBUF footprint, which
lets Q/K/V stay SBUF-resident between the projection and attention
phases (no DRAM round-trip) and doubles vector-engine throughput.
"""

import os
import sys

sys.path.insert(0, "/opt/trn_rl_repo")

import numpy as np
import ml_dtypes

BF16 = ml_dtypes.bfloat16

B = 2
S = 2048
D = 2048
H = 16
DK = 128
N_CORES = 8
HPC = 4          # heads per core
E = HPC * DK     # 512: per-core slice of the model dim
AN = 512         # phase-A sequence chunk (moving free dim for Q/K)
SC = 512         # attention query chunk (moving free dim)
KO = D // 128    # contraction chunks for the projections
NJ = S // 128    # key chunks
NI = S // SC     # query chunks
NN = S // AN     # phase-A chunks
ISQRT_DK = 1.0 / np.sqrt(DK)

_CACHE = {}

last_exec_time_ns = None
last_results = None


def _build_program():
    import concourse.mybir as mybir
    import concourse.tile as tile
    from concourse import bacc

    dt = mybir.dt
    F32 = dt.float32
    BF = dt.bfloat16
    AF = mybir.ActivationFunctionType

    nc = bacc.Bacc(None, target_bir_lowering=False, debug=True)

    # all inputs are pre-swizzled on the host so every DMA lands with one
    # contiguous >=4KB run per partition (row index = tile partition)
    xP = nc.dram_tensor("xP", [NN * 4 * 128, 4 * AN], BF, kind="ExternalInput")
    wqP = nc.dram_tensor("wqP", [4 * 128, 4 * E], BF, kind="ExternalInput")
    wkP = nc.dram_tensor("wkP", [4 * 128, 4 * E], BF, kind="ExternalInput")
    wvP = nc.dram_tensor("wvP", [4 * 128, 4 * E], BF, kind="ExternalInput")
    woP = nc.dram_tensor("woP", [128, HPC * D], BF, kind="ExternalInput")
    bq = nc.dram_tensor("bq", [HPC, DK], F32, kind="ExternalInput")
    bk = nc.dram_tensor("bk", [HPC, DK], F32, kind="ExternalInput")
    cc2 = nc.dram_tensor("cc2", [DK, S], BF, kind="ExternalInput")
    sss = nc.dram_tensor("sss", [DK, S], BF, kind="ExternalInput")
    masks = nc.dram_tensor("masks", [128, HPC * SC], BF, kind="ExternalInput")
    ones = nc.dram_tensor("ones", [128, 128], BF, kind="ExternalInput")
    # partial sums leave the device in bf16 (the host upcasts and reduces
    # in fp32); halves the write-back traffic in phase C
    out = nc.dram_tensor("out", [S, D], BF, kind="ExternalOutput")

    with tile.TileContext(nc) as tc:
        with tc.tile_pool(name="const", bufs=1) as cpool:
            # DMA arbitration is roughly fair-share across queues, so every
            # byte requested early steals bandwidth from the critical
            # startup stream (wq + x chunk 0). Deferrable loads (masks,
            # ones, wo, later x chunks) are emitted later in the issuing
            # engine's instruction stream so they only start mid-phase.
            cc2_sb = cpool.tile([DK, S], BF, name="cc2_sb")
            nc.sync.dma_start(cc2_sb[:], cc2[:])
            sss_sb = cpool.tile([DK, S], BF, name="sss_sb")
            nc.sync.dma_start(sss_sb[:], sss[:])
            mask_sb = cpool.tile([128, HPC, SC], BF, name="mask_sb")
            ones_sb = cpool.tile([128, 128], BF, name="ones_sb")

            # persistent activations: V, Q, K, attention output (all bf16)
            res_ctx = tc.tile_pool(name="resident", bufs=1)
            rpool = res_ctx.__enter__()
            vt_all = rpool.tile([128, NJ, E], BF, name="vt_all")
            q_all = rpool.tile([DK, HPC, S], BF, name="q_all")
            k_all = rpool.tile([DK, HPC, S], BF, name="k_all")
            ao_all = rpool.tile([DK, HPC, S], BF, name="ao_all")
            wo_sb = rpool.tile([128, HPC, D], BF, name="wo_sb")

            # ---------- Phase A: Q/K/V projections (+ RoPE on Q/K) ----------
            with (
                tc.tile_pool(name="aw", bufs=1) as awpool,
                tc.tile_pool(name="ax", bufs=2) as axpool,
                tc.tile_pool(name="ast", bufs=3) as astpool,
                tc.tile_pool(name="aps", bufs=2, space="PSUM") as apspool,
            ):
                # weights and x split into per-g tiles so the first matmuls
                # wait only on the first 512-row piece, not the whole tensor
                def load_w(wdram, nm, q):
                    tiles = []
                    for g in range(4):
                        t = awpool.tile([128, 4, E], BF, name=f"{nm}{g}")
                        q.dma_start(t[:], wdram[g * 128 : (g + 1) * 128, :])
                        tiles.append(t)
                    return tiles

                def load_xn(n, q):
                    tiles = []
                    for g in range(4):
                        t = axpool.tile(
                            [128, 4, AN], BF, tag=f"xn{g}", name=f"xn{n}_{g}"
                        )
                        r0 = (n * 4 + g) * 128
                        q.dma_start(t[:], xP[r0 : r0 + 128, :])
                        tiles.append(t)
                    return tiles

                # startup-critical: (wq, x0) pieces interleaved on the
                # highest-priority queue, then the phase-A biases
                wq_t = []
                x_next = []
                for g in range(4):
                    t = awpool.tile([128, 4, E], BF, name=f"wq{g}")
                    nc.gpsimd.dma_start(t[:], wqP[g * 128 : (g + 1) * 128, :])
                    wq_t.append(t)
                    xt = axpool.tile([128, 4, AN], BF, tag=f"xn{g}", name=f"xn0_{g}")
                    nc.gpsimd.dma_start(xt[:], xP[g * 128 : (g + 1) * 128, :])
                    x_next.append(xt)
                bq_sb = cpool.tile([DK, HPC], F32, name="bq_sb")
                nc.gpsimd.dma_start(bq_sb[:], bq[:].rearrange("h d -> d h"))
                bk_sb = cpool.tile([DK, HPC], F32, name="bk_sb")
                nc.gpsimd.dma_start(bk_sb[:], bk[:].rearrange("h d -> d h"))
                wk_t = load_w(wkP, "wk", nc.sync)
                wv_t = load_w(wvP, "wv", nc.sync)

                def rope_store(pq, bsb, m, dst, nsl):
                    st0 = astpool.tile([128, AN], BF, tag="qkst0")
                    nc.scalar.activation(
                        st0[:], pq[:], AF.Identity, bias=bsb[:, m : m + 1]
                    )
                    # RoPE: d-rows are packed [even; odd] per head, so
                    # rotate pairs are partition r <-> r+64
                    sw = astpool.tile([128, AN], BF, tag="qksw")
                    nc.vector.tensor_copy(sw[0:64, :], st0[64:128, :])
                    nc.vector.tensor_copy(sw[64:128, :], st0[0:64, :])
                    rot = astpool.tile([128, AN], BF, tag="qkrot")
                    nc.vector.tensor_mul(rot[:], st0[:], cc2_sb[:, nsl])
                    nc.vector.tensor_mul(sw[:], sw[:], sss_sb[:, nsl])
                    nc.vector.tensor_add(dst[:, m, nsl], rot[:], sw[:])

                for n in range(NN):
                    xn = x_next
                    nsl = slice(n * AN, (n + 1) * AN)
                    # need-gated deferred loads: the scalar engine only
                    # reaches these issue points mid-phase, keeping early
                    # HBM bandwidth for the critical stream
                    if n == 2:
                        nc.scalar.dma_start(mask_sb[:], masks[:])
                        nc.scalar.dma_start(ones_sb[:], ones[:])
                    if n == 3:
                        nc.scalar.dma_start(wo_sb[:], woP[:])
                    # Q and K: out[d, s], then bias + RoPE (result written
                    # straight into the resident SBUF q/k tiles).
                    # First chunk runs k-outer so the PE consumes each
                    # weight/x piece as it arrives instead of stalling.
                    for wi, (wt, bsb, dst) in enumerate(
                        ((wq_t, bq_sb, q_all), (wk_t, bk_sb, k_all))
                    ):
                        if n == 0:
                            pqs = [
                                apspool.tile(
                                    [128, AN], F32, tag="pqk", bufs=4,
                                    name=f"pq0_{m}",
                                )
                                for m in range(HPC)
                            ]
                            for k in range(KO):
                                for m in range(HPC):
                                    nc.tensor.matmul(
                                        pqs[m][:],
                                        wt[k // 4][:, k % 4, m * DK : (m + 1) * DK],
                                        xn[k // 4][:, k % 4, :],
                                        start=(k == 0),
                                        stop=(k == KO - 1),
                                    )
                            for m in range(HPC):
                                rope_store(pqs[m], bsb, m, dst, nsl)
                        else:
                            for m in range(HPC):
                                pq = apspool.tile([128, AN], F32, tag="pqk", bufs=4)
                                for k in range(KO):
                                    nc.tensor.matmul(
                                        pq[:],
                                        wt[k // 4][:, k % 4, m * DK : (m + 1) * DK],
                                        xn[k // 4][:, k % 4, :],
                                        start=(k == 0),
                                        stop=(k == KO - 1),
                                    )
                                rope_store(pq, bsb, m, dst, nsl)
                        if wi == 0 and n + 1 < NN:
                            # next x chunk: issued by the scalar engine only
                            # after this chunk's Q activations have started
                            x_next = load_xn(n + 1, nc.scalar)
                    # V: out[s, d] with s on partitions (natural for P@V)
                    for jj in range(AN // 128):
                        pv = apspool.tile([128, E], F32, tag="pv")
                        for k in range(KO):
                            nc.tensor.matmul(
                                pv[:],
                                xn[k // 4][:, k % 4, jj * 128 : (jj + 1) * 128],
                                wv_t[k // 4][:, k % 4, :],
                                start=(k == 0),
                                stop=(k == KO - 1),
                            )
                        nc.vector.tensor_copy(vt_all[:, n * 4 + jj, :], pv[:])

            # ---------- Phase B: causal attention per head ----------
            # scores land in paired PSUM tiles [128, 2, SC] so one exp
            # instruction covers two key-chunks (amortizes ACT overhead);
            # the softmax row-sum rides the tensor engine (ones matmul).
            # A software pipeline carried across (head, ic) iterations keeps
            # the PE from draining at chunk boundaries.
            with (
                tc.tile_pool(name="bp", bufs=6) as bp,
                tc.tile_pool(name="bli", bufs=2) as bli,
                tc.tile_pool(name="bps_s", bufs=2, space="PSUM") as bps_s,
                tc.tile_pool(name="bps_o", bufs=2, space="PSUM") as bps_o,
                tc.tile_pool(name="bps_l", bufs=2, space="PSUM") as bps_l,
            ):
                pending = []  # (p2, half, jc, cs, po, pl, njc, fin)
                DEPTH = 3

                def emit_pv(p2, half, jc, cs, po, pl, njc, fin):
                    h0 = fin[0]
                    nc.tensor.matmul(
                        po[:, cs:],
                        vt_all[:, jc, h0 * DK : (h0 + 1) * DK],
                        p2[:, half, cs:],
                        start=(jc == 0),
                        stop=(jc == njc - 1),
                    )
                    nc.tensor.matmul(
                        pl[:, cs:],
                        ones_sb[:],
                        p2[:, half, cs:],
                        start=(jc == 0),
                        stop=(jc == njc - 1),
                    )
                    if jc == njc - 1:
                        # normalization for this (head, ic) now that the
                        # last accumulating matmul is emitted
                        _, i0 = fin
                        li = bli.tile([128, SC], F32, tag="li")
                        nc.vector.reciprocal_approx_fast(li[:], pl[:])
                        nc.vector.tensor_mul(
                            ao_all[:, h0, i0 : i0 + SC], po[:], li[:]
                        )

                for h0 in range(HPC):
                    for ic in range(NI):
                        po = bps_o.tile([128, SC], F32, tag="po")
                        pl = bps_l.tile([128, SC], F32, tag="pl")
                        njc = 4 * ic + 4
                        i0 = ic * SC
                        fin = (h0, i0)

                        for jp in range(njc // 2):
                            ps = bps_s.tile([128, 2, SC], F32, tag="ps")
                            p2 = bp.tile([128, 2, SC], BF, tag="p")
                            css = []
                            for half in range(2):
                                jc = 2 * jp + half
                                t = jc - 4 * ic  # >=0 on the diagonal band
                                cs = 128 * t if t >= 0 else 0
                                css.append((jc, t, cs))
                                nc.tensor.matmul(
                                    ps[:, half, cs:],
                                    k_all[:, h0, jc * 128 : (jc + 1) * 128],
                                    q_all[:, h0, i0 + cs : i0 + SC],
                                    start=True,
                                    stop=True,
                                )
                            t0_, t1_ = css[0][1], css[1][1]
                            if t1_ <= 1:
                                # both halves (nearly) full: one wide exp
                                nc.scalar.activation(
                                    p2[:], ps[:], AF.Exp, scale=float(ISQRT_DK)
                                )
                            else:
                                for half, (jc, t, cs) in enumerate(css):
                                    nc.scalar.activation(
                                        p2[:, half, cs:],
                                        ps[:, half, cs:],
                                        AF.Exp,
                                        scale=float(ISQRT_DK),
                                    )
                            for half, (jc, t, cs) in enumerate(css):
                                if t >= 0:
                                    nc.vector.tensor_mul(
                                        p2[:, half, cs : cs + 128],
                                        p2[:, half, cs : cs + 128],
                                        mask_sb[:, t, cs : cs + 128],
                                    )
                                pending.append((p2, half, jc, cs, po, pl, njc, fin))
                                if len(pending) > DEPTH:
                                    emit_pv(*pending.pop(0))
                for it in pending:
                    emit_pv(*it)
                pending.clear()

            # ---------- Phase C: output projection (partial sum) ----------
            # PSUM -> SBUF staging copies spread over three engines; two
            # queues share the write-back stream
            with (
                tc.tile_pool(name="cst", bufs=3) as cst,
                tc.tile_pool(name="cps", bufs=8, space="PSUM") as cps,
            ):
                for ii in range(S // 128):
                    pcs = [
                        cps.tile([128, 512], F32, tag="pc", name=f"pc_{ii}_{fc}")
                        for fc in range(4)
                    ]
                    for ec in range(HPC):
                        for fc in range(4):
                            nc.tensor.matmul(
                                pcs[fc][:],
                                ao_all[:, ec, ii * 128 : (ii + 1) * 128],
                                wo_sb[:, ec, fc * 512 : (fc + 1) * 512],
                                start=(ec == 0),
                                stop=(ec == HPC - 1),
                            )
                    # one wide staging tile -> a single contiguous 512KB
                    # write-back per row chunk (4KB per partition)
                    ob = cst.tile([128, 4, 512], BF, tag="ob")
                    for fc in range(4):
                        if fc % 2 == 1:
                            nc.scalar.activation(ob[:, fc, :], pcs[fc][:], AF.Copy)
                        else:
                            nc.vector.tensor_copy(ob[:, fc, :], pcs[fc][:])
                    q = (nc.sync, nc.scalar, nc.gpsimd)[ii % 3]
                    q.dma_start(out[ii * 128 : (ii + 1) * 128, :], ob[:])

            res_ctx.__exit__(None, None, None)

    nc.compile()
    return nc


def _rope_tables():
    inv_freq = 1.0 / (10000.0 ** (np.arange(0, DK, 2, dtype=np.float64) / DK))
    pos = np.arange(S, dtype=np.float64)
    freqs = pos[:, None] * inv_freq[None, :]  # [S, DK/2]
    cos_t = np.cos(freqs).T.astype(np.float32)  # [64, S]
    sin_t = np.sin(freqs).T.astype(np.float32)
    cc2 = np.ascontiguousarray(np.concatenate([cos_t, cos_t], axis=0))
    sss = np.ascontiguousarray(np.concatenate([-sin_t, sin_t], axis=0))
    return cc2, sss


def kernel(
    x, wq_w, wq_b, wk_w, wk_b, wv_w, wv_b, wo_w, wo_b
) -> np.ndarray:
    global last_exec_time_ns, last_results
    from concourse.bass_utils import run_bass_kernel_spmd

    if "nc" not in _CACHE:
        _CACHE["nc"] = _build_program()
    nc = _CACHE["nc"]

    x = np.asarray(x, dtype=np.float32)
    wq_w = np.asarray(wq_w, dtype=np.float32)
    wk_w = np.asarray(wk_w, dtype=np.float32)
    wv_w = np.asarray(wv_w, dtype=np.float32)
    wo_w = np.asarray(wo_w, dtype=np.float32)
    wq_b = np.asarray(wq_b, dtype=np.float32)
    wk_b = np.asarray(wk_b, dtype=np.float32)
    wv_b = np.asarray(wv_b, dtype=np.float32)
    wo_b = np.asarray(wo_b, dtype=np.float32)

    cc2, sss = _rope_tables()
    r_idx = np.arange(128)[:, None]
    c_idx = np.arange(SC)[None, :]
    masks = np.ascontiguousarray(
        np.stack(
            [(r_idx <= c_idx - t * 128).astype(np.float32) for t in range(HPC)]
        )
        .transpose(1, 0, 2)
        .reshape(128, HPC * SC)
    ).astype(BF16)
    ones = np.ones((128, 128), dtype=BF16)
    # within each head, pack d-rows as [even dims; odd dims]
    perm = np.concatenate([np.arange(0, DK, 2), np.arange(1, DK, 2)])

    def swz_in(mT):
        # [D, cols] -> rows (g*128+p) x cols (ko*cols): one contiguous run
        # per SBUF partition for each 512-row piece
        cols = mT.shape[1]
        return np.ascontiguousarray(
            mT.reshape(4, 4, 128, cols).transpose(0, 2, 1, 3).reshape(512, 4 * cols)
        ).astype(BF16)

    # x: [NN*4*128, 4*AN] with row ((n*4+g)*128+p), col (ko*AN+s)
    xP_b = []
    for b in range(B):
        xT = x[b].T  # [D, S]
        xP = (
            xT.reshape(4, 4, 128, NN, AN)  # [g, ko, p, n, s]
            .transpose(3, 0, 2, 1, 4)  # [n, g, p, ko, s]
            .reshape(NN * 4 * 128, 4 * AN)
        )
        xP_b.append(np.ascontiguousarray(xP).astype(BF16))
    cc2 = cc2.astype(BF16)
    sss = sss.astype(BF16)

    in_maps = []
    for c in range(N_CORES):
        b = c // (N_CORES // B)
        g = c % (N_CORES // B)
        es = g * E

        def pack_qk(w):
            rows = w[es : es + E]  # [E, D]
            blocks = [
                rows[h0 * DK : (h0 + 1) * DK][perm] for h0 in range(HPC)
            ]
            return swz_in(np.concatenate(blocks, axis=0).T)

        def pack_bias(bvec):
            sl = bvec[es : es + E].reshape(HPC, DK)
            return np.ascontiguousarray(sl[:, perm])

        woP = np.ascontiguousarray(
            wo_w[:, es : es + E].T.reshape(HPC, 128, D).transpose(1, 0, 2)
            .reshape(128, HPC * D)
        ).astype(BF16)

        in_maps.append(
            {
                "xP": xP_b[b],
                "wqP": pack_qk(wq_w),
                "wkP": pack_qk(wk_w),
                "wvP": swz_in(wv_w[es : es + E].T),
                "woP": woP,
                "bq": pack_bias(wq_b),
                "bk": pack_bias(wk_b),
                "cc2": cc2,
                "sss": sss,
                "masks": masks,
                "ones": ones,
            }
        )

    trace = bool(os.environ.get("MHA_TRACE"))
    res = run_bass_kernel_spmd(
        nc, in_maps, list(range(N_CORES)), trace=trace
    )
    last_exec_time_ns = res.exec_time_ns
    last_results = res

    # host-side gather: sum partials per batch, add biases that commute
    # with attention (softmax rows sum to 1, so wv_b passes straight
    # through to the output projection)
    const_bias = wo_b + wo_w @ wv_b  # [D]
    out = np.empty((B, S, D), dtype=np.float32)
    gpb = N_CORES // B
    for b in range(B):
        acc = res.results[b * gpb]["out"].astype(np.float32)
        for c in range(b * gpb + 1, (b + 1) * gpb):
            acc += res.results[c]["out"].astype(np.float32)
        out[b] = acc + const_bias[None, :]
    return out


# revision 34
# speedup vs baseline: 1.0179x; 1.0179x over previous
"""Trainium2 Bass kernel for causal multi-head attention with RoPE.

Full-input contract: kernel(**inputs) takes the unsharded tensors and
returns the full [B, S, D] output. Internally the work is sharded over
8 NeuronCores: cores 0-3 compute batch 0, cores 4-7 batch 1; within a
batch group each core owns 4 of the 16 heads (tensor-parallel over
heads). Each core computes its partial output-projection contribution
[S, D]; the host sums the 4 partials per batch and adds the biases
that commute with attention (wo_b, and wv_b which passes through the
softmax untouched because attention weights sum to 1).

The on-device data path is bf16 (fp32 PSUM accumulation): same PE
rate as fp32r (1 row/cycle) but half the DMA/SBUF footprint, which
lets Q/K/V stay SBUF-resident between the projection and attention
phases (no DRAM round-trip) and doubles vector-engine throughput.
"""

import os
import sys

sys.path.insert(0, "/opt/trn_rl_repo")

import numpy as np
import ml_dtypes

BF16 = ml_dtypes.bfloat16

B = 2
S = 2048
D = 2048
H = 16
DK = 128
N_CORES = 8
HPC = 4          # heads per core
E = HPC * DK     # 512: per-core slice of the model dim
AN = 512         # phase-A sequence chunk (moving free dim for Q/K)
SC = 512         # attention query chunk (moving free dim)
KO = D // 128    # contraction chunks for the projections
NJ = S // 128    # key chunks
NI = S // SC     # query chunks
NN = S // AN     # phase-A chunks
ISQRT_DK = 1.0 / np.sqrt(DK)

_CACHE = {}

last_exec_time_ns = None
last_results = None


def _build_program():
    import concourse.mybir as mybir
    import concourse.tile as tile
    from concourse import bacc

    dt = mybir.dt
    F32 = dt.float32
    BF = dt.bfloat16
    AF = mybir.ActivationFunctionType

    nc = bacc.Bacc(None, target_bir_lowering=False, debug=True)

    # all inputs are pre-swizzled on the host so every DMA lands with one
    # contiguous >=4KB run per partition (row index = tile partition)
    xP = nc.dram_tensor("xP", [NN * 4 * 128, 4 * AN], BF, kind="ExternalInput")
    wqP = nc.dram_tensor("wqP", [4 * 128, 4 * E], BF, kind="ExternalInput")
    wkP = nc.dram_tensor("wkP", [4 * 128, 4 * E], BF, kind="ExternalInput")
    wvP = nc.dram_tensor("wvP", [4 * 128, 4 * E], BF, kind="ExternalInput")
    woP = nc.dram_tensor("woP", [128, HPC * D], BF, kind="ExternalInput")
    bq = nc.dram_tensor("bq", [HPC, DK], F32, kind="ExternalInput")
    bk = nc.dram_tensor("bk", [HPC, DK], F32, kind="ExternalInput")
    cc2 = nc.dram_tensor("cc2", [DK, S], BF, kind="ExternalInput")
    sss = nc.dram_tensor("sss", [DK, S], BF, kind="ExternalInput")
    masks = nc.dram_tensor("masks", [128, HPC * SC], BF, kind="ExternalInput")
    ones = nc.dram_tensor("ones", [128, 128], BF, kind="ExternalInput")
    # partial sums leave the device in bf16 (the host upcasts and reduces
    # in fp32); halves the write-back traffic in phase C
    out = nc.dram_tensor("out", [S, D], BF, kind="ExternalOutput")

    with tile.TileContext(nc) as tc:
        with tc.tile_pool(name="const", bufs=1) as cpool:
            # DMA arbitration is roughly fair-share across queues, so every
            # byte requested early steals bandwidth from the critical
            # startup stream (wq + x chunk 0). Deferrable loads (masks,
            # ones, wo, later x chunks) are emitted later in the issuing
            # engine's instruction stream so they only start mid-phase.
            cc2_sb = cpool.tile([DK, S], BF, name="cc2_sb")
            nc.sync.dma_start(cc2_sb[:], cc2[:])
            sss_sb = cpool.tile([DK, S], BF, name="sss_sb")
            nc.sync.dma_start(sss_sb[:], sss[:])
            mask_sb = cpool.tile([128, HPC, SC], BF, name="mask_sb")
            ones_sb = cpool.tile([128, 128], BF, name="ones_sb")

            # persistent activations: V, Q, K, attention output (all bf16)
            res_ctx = tc.tile_pool(name="resident", bufs=1)
            rpool = res_ctx.__enter__()
            vt_all = rpool.tile([128, NJ, E], BF, name="vt_all")
            q_all = rpool.tile([DK, HPC, S], BF, name="q_all")
            k_all = rpool.tile([DK, HPC, S], BF, name="k_all")
            ao_all = rpool.tile([DK, HPC, S], BF, name="ao_all")
            wo_sb = rpool.tile([128, HPC, D], BF, name="wo_sb")

            # ---------- Phase A: Q/K/V projections (+ RoPE on Q/K) ----------
            with (
                tc.tile_pool(name="aw", bufs=1) as awpool,
                tc.tile_pool(name="ax", bufs=2) as axpool,
                tc.tile_pool(name="ast", bufs=3) as astpool,
                tc.tile_pool(name="aps", bufs=2, space="PSUM") as apspool,
            ):
                # weights and x split into per-g tiles so the first matmuls
                # wait only on the first 512-row piece, not the whole tensor
                def load_w(wdram, nm, q):
                    tiles = []
                    for g in range(4):
                        t = awpool.tile([128, 4, E], BF, name=f"{nm}{g}")
                        q.dma_start(t[:], wdram[g * 128 : (g + 1) * 128, :])
                        tiles.append(t)
                    return tiles

                def load_xn(n, q):
                    tiles = []
                    for g in range(4):
                        t = axpool.tile(
                            [128, 4, AN], BF, tag=f"xn{g}", name=f"xn{n}_{g}"
                        )
                        r0 = (n * 4 + g) * 128
                        q.dma_start(t[:], xP[r0 : r0 + 128, :])
                        tiles.append(t)
                    return tiles

                # Three passes (all Q, all K, all V), re-streaming x once per
                # pass: the startup window then only needs wq + the first x
                # chunk, and the x tag ring (bufs=2) self-paces every later
                # chunk's DMA just-in-time behind its consumer.
                # Queues: gpsimd carries wq + biases; sync carries x + rope
                # tables; scalar (paced by this engine's activation work)
                # carries wk, wv, masks, and wo at need-gated points.
                wq_t = []
                for g in range(4):
                    t = awpool.tile([128, 4, E], BF, name=f"wq{g}")
                    nc.gpsimd.dma_start(t[:], wqP[g * 128 : (g + 1) * 128, :])
                    wq_t.append(t)
                bq_sb = cpool.tile([DK, HPC], F32, name="bq_sb")
                nc.gpsimd.dma_start(bq_sb[:], bq[:].rearrange("h d -> d h"))
                bk_sb = cpool.tile([DK, HPC], F32, name="bk_sb")
                nc.gpsimd.dma_start(bk_sb[:], bk[:].rearrange("h d -> d h"))
                x_next = load_xn(0, nc.sync)
                nc.sync.dma_start(cc2_sb[:], cc2[:])
                nc.sync.dma_start(sss_sb[:], sss[:])
                wk_t = [
                    awpool.tile([128, 4, E], BF, name=f"wk{g}") for g in range(4)
                ]
                wv_t = [
                    awpool.tile([128, 4, E], BF, name=f"wv{g}") for g in range(4)
                ]

                def rope_store(pq, bsb, m, dst, nsl):
                    st0 = astpool.tile([128, AN], BF, tag="qkst0")
                    nc.scalar.activation(
                        st0[:], pq[:], AF.Identity, bias=bsb[:, m : m + 1]
                    )
                    # RoPE: d-rows are packed [even; odd] per head, so
                    # rotate pairs are partition r <-> r+64
                    sw = astpool.tile([128, AN], BF, tag="qksw")
                    nc.vector.tensor_copy(sw[0:64, :], st0[64:128, :])
                    nc.vector.tensor_copy(sw[64:128, :], st0[0:64, :])
                    rot = astpool.tile([128, AN], BF, tag="qkrot")
                    nc.vector.tensor_mul(rot[:], st0[:], cc2_sb[:, nsl])
                    nc.vector.tensor_mul(sw[:], sw[:], sss_sb[:, nsl])
                    nc.vector.tensor_add(dst[:, m, nsl], rot[:], sw[:])

                def qk_pass(wt, bsb, dst, k_outer_first, c0, defer):
                    nonlocal x_next
                    for n in range(NN):
                        xn = x_next
                        if c0 + n + 1 < 3 * NN:
                            x_next = load_xn((c0 + n + 1) % NN, nc.sync)
                        if n in defer:
                            defer[n]()
                        nsl = slice(n * AN, (n + 1) * AN)
                        if n == 0 and k_outer_first:
                            pqs = [
                                apspool.tile(
                                    [128, AN], F32, tag="pqk", bufs=4,
                                    name=f"pq0_{m}",
                                )
                                for m in range(HPC)
                            ]
                            for k in range(KO):
                                for m in range(HPC):
                                    nc.tensor.matmul(
                                        pqs[m][:],
                                        wt[k // 4][:, k % 4, m * DK : (m + 1) * DK],
                                        xn[k // 4][:, k % 4, :],
                                        start=(k == 0),
                                        stop=(k == KO - 1),
                                    )
                            for m in range(HPC):
                                rope_store(pqs[m], bsb, m, dst, nsl)
                        else:
                            for m in range(HPC):
                                pq = apspool.tile([128, AN], F32, tag="pqk", bufs=4)
                                for k in range(KO):
                                    nc.tensor.matmul(
                                        pq[:],
                                        wt[k // 4][:, k % 4, m * DK : (m + 1) * DK],
                                        xn[k // 4][:, k % 4, :],
                                        start=(k == 0),
                                        stop=(k == KO - 1),
                                    )
                                rope_store(pq, bsb, m, dst, nsl)

                def emit_w(tiles, wdram):
                    def go():
                        for g in range(4):
                            nc.scalar.dma_start(
                                tiles[g][:], wdram[g * 128 : (g + 1) * 128, :]
                            )
                    return go

                def emit_bc():
                    nc.scalar.dma_start(mask_sb[:], masks[:])
                    nc.scalar.dma_start(ones_sb[:], ones[:])
                    nc.scalar.dma_start(wo_sb[:], woP[:])

                qk_pass(wq_t, bq_sb, q_all, True, 0, {1: emit_w(wk_t, wkP)})
                qk_pass(wk_t, bk_sb, k_all, False, NN, {1: emit_w(wv_t, wvP)})

                # ---- V pass ----
                for n in range(NN):
                    xn = x_next
                    if 2 * NN + n + 1 < 3 * NN:
                        x_next = load_xn(n + 1, nc.sync)
                    if n == 1:
                        emit_bc()
                    # V: out[s, d] with s on partitions (natural for P@V)
                    for jj in range(AN // 128):
                        pv = apspool.tile([128, E], F32, tag="pv")
                        for k in range(KO):
                            nc.tensor.matmul(
                                pv[:],
                                xn[k // 4][:, k % 4, jj * 128 : (jj + 1) * 128],
                                wv_t[k // 4][:, k % 4, :],
                                start=(k == 0),
                                stop=(k == KO - 1),
                            )
                        nc.vector.tensor_copy(vt_all[:, n * 4 + jj, :], pv[:])

            # ---------- Phase B: causal attention per head ----------
            # scores land in paired PSUM tiles [128, 2, SC] so one exp
            # instruction covers two key-chunks (amortizes ACT overhead);
            # the softmax row-sum rides the tensor engine (ones matmul).
            # A software pipeline carried across (head, ic) iterations keeps
            # the PE from draining at chunk boundaries. The last head (h=3)
            # runs with unpaired 1-bank score tiles so the output projection
            # (phase C) interleaves with it chunk-by-chunk: C's dense
            # matmuls fill h3's exp-latency bubbles and the write-back
            # stream starts ~20us earlier.
            b_outer = (
                tc.tile_pool(name="bp", bufs=6),
                tc.tile_pool(name="bli", bufs=2),
                tc.tile_pool(name="bps_o", bufs=2, space="PSUM"),
                tc.tile_pool(name="bps_l", bufs=2, space="PSUM"),
            )
            bp = b_outer[0].__enter__()
            bli = b_outer[1].__enter__()
            bps_o = b_outer[2].__enter__()
            bps_l = b_outer[3].__enter__()

            pending = []  # (p2, half, jc, cs, po, pl, njc, fin)
            DEPTH = 3

            def emit_pv(p2, half, jc, cs, po, pl, njc, fin):
                h0 = fin[0]
                nc.tensor.matmul(
                    po[:, cs:],
                    vt_all[:, jc, h0 * DK : (h0 + 1) * DK],
                    p2[:, half, cs:],
                    start=(jc == 0),
                    stop=(jc == njc - 1),
                )
                nc.tensor.matmul(
                    pl[:, cs:],
                    ones_sb[:],
                    p2[:, half, cs:],
                    start=(jc == 0),
                    stop=(jc == njc - 1),
                )
                if jc == njc - 1:
                    # normalization for this (head, ic) now that the
                    # last accumulating matmul is emitted
                    _, i0 = fin
                    li = bli.tile([128, SC], F32, tag="li")
                    nc.vector.reciprocal_approx_fast(li[:], pl[:])
                    nc.vector.tensor_mul(
                        ao_all[:, h0, i0 : i0 + SC], po[:], li[:]
                    )

            def attn_chunk(h0, ic, bps_s, paired):
                po = bps_o.tile([128, SC], F32, tag="po")
                pl = bps_l.tile([128, SC], F32, tag="pl")
                njc = 4 * ic + 4
                i0 = ic * SC
                fin = (h0, i0)
                width = 2 if paired else 1

                for jp in range(njc // width):
                    ps = bps_s.tile([128, width, SC], F32, tag="ps")
                    p2 = bp.tile([128, width, SC], BF, tag="p")
                    css = []
                    for half in range(width):
                        jc = width * jp + half
                        t = jc - 4 * ic  # >=0 on the diagonal band
                        cs = 128 * t if t >= 0 else 0
                        css.append((jc, t, cs))
                        nc.tensor.matmul(
                            ps[:, half, cs:],
                            k_all[:, h0, jc * 128 : (jc + 1) * 128],
                            q_all[:, h0, i0 + cs : i0 + SC],
                            start=True,
                            stop=True,
                        )
                    if css[-1][1] <= 1:
                        # all halves (nearly) full: one wide exp
                        nc.scalar.activation(
                            p2[:], ps[:], AF.Exp, scale=float(ISQRT_DK)
                        )
                    else:
                        for half, (jc, t, cs) in enumerate(css):
                            nc.scalar.activation(
                                p2[:, half, cs:],
                                ps[:, half, cs:],
                                AF.Exp,
                                scale=float(ISQRT_DK),
                            )
                    for half, (jc, t, cs) in enumerate(css):
                        if t >= 0:
                            nc.vector.tensor_mul(
                                p2[:, half, cs : cs + 128],
                                p2[:, half, cs : cs + 128],
                                mask_sb[:, t, cs : cs + 128],
                            )
                        pending.append((p2, half, jc, cs, po, pl, njc, fin))
                        if len(pending) > DEPTH:
                            emit_pv(*pending.pop(0))

            def flush():
                for it in pending:
                    emit_pv(*it)
                pending.clear()

            with tc.tile_pool(name="bps_s2", bufs=2, space="PSUM") as bps_s2:
                for h0 in range(HPC - 1):
                    for ic in range(NI):
                        attn_chunk(h0, ic, bps_s2, True)
                flush()

            # ---------- h3 + phase C interleaved ----------
            with (
                tc.tile_pool(name="bps_s1", bufs=2, space="PSUM") as bps_s1,
                tc.tile_pool(name="cst", bufs=3) as cst,
                tc.tile_pool(name="cps", bufs=2, space="PSUM") as cps,
            ):
                for ic in range(NI):
                    attn_chunk(HPC - 1, ic, bps_s1, False)
                    flush()
                    for ii in range(4 * ic, 4 * ic + 4):
                        ob = cst.tile([128, 4, 512], BF, tag="ob")
                        for fc in range(4):
                            pc = cps.tile([128, 512], F32, tag="pc")
                            for ec in range(HPC):
                                nc.tensor.matmul(
                                    pc[:],
                                    ao_all[:, ec, ii * 128 : (ii + 1) * 128],
                                    wo_sb[:, ec, fc * 512 : (fc + 1) * 512],
                                    start=(ec == 0),
                                    stop=(ec == HPC - 1),
                                )
                            if fc % 2 == 1:
                                nc.scalar.activation(ob[:, fc, :], pc[:], AF.Copy)
                            else:
                                nc.vector.tensor_copy(ob[:, fc, :], pc[:])
                        q = (nc.sync, nc.scalar, nc.gpsimd)[ii % 3]
                        q.dma_start(out[ii * 128 : (ii + 1) * 128, :], ob[:])

            for p in reversed(b_outer):
                p.__exit__(None, None, None)

            res_ctx.__exit__(None, None, None)

    nc.compile()
    return nc


def _rope_tables():
    inv_freq = 1.0 / (10000.0 ** (np.arange(0, DK, 2, dtype=np.float64) / DK))
    pos = np.arange(S, dtype=np.float64)
    freqs = pos[:, None] * inv_freq[None, :]  # [S, DK/2]
    cos_t = np.cos(freqs).T.astype(np.float32)  # [64, S]
    sin_t = np.sin(freqs).T.astype(np.float32)
    cc2 = np.ascontiguousarray(np.concatenate([cos_t, cos_t], axis=0))
    sss = np.ascontiguousarray(np.concatenate([-sin_t, sin_t], axis=0))
    return cc2, sss


def kernel(
    x, wq_w, wq_b, wk_w, wk_b, wv_w, wv_b, wo_w, wo_b
) -> np.ndarray:
    global last_exec_time_ns, last_results
    from concourse.bass_utils import run_bass_kernel_spmd

    if "nc" not in _CACHE:
        _CACHE["nc"] = _build_program()
    nc = _CACHE["nc"]

    x = np.asarray(x, dtype=np.float32)
    wq_w = np.asarray(wq_w, dtype=np.float32)
    wk_w = np.asarray(wk_w, dtype=np.float32)
    wv_w = np.asarray(wv_w, dtype=np.float32)
    wo_w = np.asarray(wo_w, dtype=np.float32)
    wq_b = np.asarray(wq_b, dtype=np.float32)
    wk_b = np.asarray(wk_b, dtype=np.float32)
    wv_b = np.asarray(wv_b, dtype=np.float32)
    wo_b = np.asarray(wo_b, dtype=np.float32)

    cc2, sss = _rope_tables()
    r_idx = np.arange(128)[:, None]
    c_idx = np.arange(SC)[None, :]
    masks = np.ascontiguousarray(
        np.stack(
            [(r_idx <= c_idx - t * 128).astype(np.float32) for t in range(HPC)]
        )
        .transpose(1, 0, 2)
        .reshape(128, HPC * SC)
    ).astype(BF16)
    ones = np.ones((128, 128), dtype=BF16)
    # within each head, pack d-rows as [even dims; odd dims]
    perm = np.concatenate([np.arange(0, DK, 2), np.arange(1, DK, 2)])

    def swz_in(mT):
        # [D, cols] -> rows (g*128+p) x cols (ko*cols): one contiguous run
        # per SBUF partition for each 512-row piece
        cols = mT.shape[1]
        return np.ascontiguousarray(
            mT.reshape(4, 4, 128, cols).transpose(0, 2, 1, 3).reshape(512, 4 * cols)
        ).astype(BF16)

    # x: [NN*4*128, 4*AN] with row ((n*4+g)*128+p), col (ko*AN+s)
    xP_b = []
    for b in range(B):
        xT = x[b].T  # [D, S]
        xP = (
            xT.reshape(4, 4, 128, NN, AN)  # [g, ko, p, n, s]
            .transpose(3, 0, 2, 1, 4)  # [n, g, p, ko, s]
            .reshape(NN * 4 * 128, 4 * AN)
        )
        xP_b.append(np.ascontiguousarray(xP).astype(BF16))
    cc2 = cc2.astype(BF16)
    sss = sss.astype(BF16)

    in_maps = []
    for c in range(N_CORES):
        b = c // (N_CORES // B)
        g = c % (N_CORES // B)
        es = g * E

        def pack_qk(w):
            rows = w[es : es + E]  # [E, D]
            blocks = [
                rows[h0 * DK : (h0 + 1) * DK][perm] for h0 in range(HPC)
            ]
            return swz_in(np.concatenate(blocks, axis=0).T)

        def pack_bias(bvec):
            sl = bvec[es : es + E].reshape(HPC, DK)
            return np.ascontiguousarray(sl[:, perm])

        woP = np.ascontiguousarray(
            wo_w[:, es : es + E].T.reshape(HPC, 128, D).transpose(1, 0, 2)
            .reshape(128, HPC * D)
        ).astype(BF16)

        in_maps.append(
            {
                "xP": xP_b[b],
                "wqP": pack_qk(wq_w),
                "wkP": pack_qk(wk_w),
                "wvP": swz_in(wv_w[es : es + E].T),
                "woP": woP,
                "bq": pack_bias(wq_b),
                "bk": pack_bias(wk_b),
                "cc2": cc2,
                "sss": sss,
                "masks": masks,
                "ones": ones,
            }
        )

    trace = bool(os.environ.get("MHA_TRACE"))
    res = run_bass_kernel_spmd(
        nc, in_maps, list(range(N_CORES)), trace=trace
    )
    last_exec_time_ns = res.exec_time_ns
    last_results = res

    # host-side gather: sum partials per batch, add biases that commute
    # with attention (softmax rows sum to 1, so wv_b passes straight
    # through to the output projection)
    const_bias = wo_b + wo_w @ wv_b  # [D]
    out = np.empty((B, S, D), dtype=np.float32)
    gpb = N_CORES // B
    for b in range(B):
        acc = res.results[b * gpb]["out"].astype(np.float32)
        for c in range(b * gpb + 1, (b + 1) * gpb):
            acc += res.results[c]["out"].astype(np.float32)
        out[b] = acc + const_bias[None, :]
    return out
